# revision 6
# baseline (speedup 1.0000x reference)
"""VQ codebook (vector-quantization) Trainium2 kernel.

Data-parallel over 8 NeuronCores: each core takes 2 of the 16 batches.
Per core: scores s[t,k] = 2*z_t.e_k - |e_k|^2 via fp32r PE matmuls (bias row
folded in as a K=1 matmul), argmax per token via DVE max/max_index, codebook
row gather via indirect DMA, and PE transposes to produce the NCHW outputs.

Returns (z_out, z_q, zp) matching the reference:
  zp    = transpose(z)  [B,H,W,C]
  z_q   = embedding[argmin_k ||z - e_k||^2]  [B,H,W,C]
  z_out = z_q laid out as NCHW (straight-through estimator is numerically z_q)
"""
import json
import numpy as np

B, C, H, W = 16, 256, 64, 64
K = 1024
NCORES = 8
BPC = B // NCORES  # batches per core
HW = H * W
P = 128
TPB = HW // P  # token tiles per batch
N_TILES = BPC * TPB  # token tiles per core

_BUILT = None


def _install_waitsplit():
    """walrus in this toolchain accepts at most ONE sync wait per instruction;
    split extra waits onto standalone EventSemaphore instructions."""
    from concourse.bass import Bass

    if getattr(Bass, "_waitsplit_installed", False):
        return
    Bass._waitsplit_installed = True
    orig = Bass.to_json_bytes

    def _split(bir):
        n = 0
        for fn in bir.get("functions", []):
            for blk in fn.get("blocks", []):
                out = []
                for ins in blk.get("instructions", []):
                    si = ins.get("sync_info")
                    ow = (si or {}).get("on_wait") or []
                    if len(ow) > 1:
                        for j, w in enumerate(ow[:-1]):
                            n += 1
                            out.append(
                                {
                                    "name": f"{ins['name']}-ws{j}",
                                    "opcode": "EventSemaphore",
                                    "engine": ins["engine"],
                                    "ins": [],
                                    "outs": [],
                                    "sync_info": {"on_wait": [w], "on_update": []},
                                }
                            )
                        si["on_wait"] = [ow[-1]]
                    out.append(ins)
                blk["instructions"] = out
        return n

    def patched(self, *a, **kw):
        m = json.loads(orig(self, *a, **kw))
        _split(m)
        return json.dumps(m).encode()

    Bass.to_json_bytes = patched


def build(n_tiles=N_TILES):
    import concourse.bass as bass
    import concourse.mybir as mybir
    import concourse.tile as tile

    _install_waitsplit()
    f32 = mybir.dt.float32
    f32r = mybir.dt.float32r
    u32 = mybir.dt.uint32

    nc = bass.Bass()
    z = nc.dram_tensor("z", [BPC, C, HW], f32r, kind="ExternalInput")
    etp = nc.dram_tensor("etp", [4, P, K], f32r, kind="ExternalInput")
    etab = nc.dram_tensor("etab", [K, C], f32, kind="ExternalInput")
    ident_d = nc.dram_tensor("ident", [P, P], f32r, kind="ExternalInput")
    zq_o = nc.dram_tensor("zq", [BPC, HW, C], f32, kind="ExternalOutput")
    zo_o = nc.dram_tensor("zo", [BPC, C, HW], f32, kind="ExternalOutput")
    zp_o = nc.dram_tensor("zp", [BPC, HW, C], f32, kind="ExternalOutput")

    with tile.TileContext(nc) as tc:
        with (
            tc.tile_pool(name="const", bufs=1) as cst,
            tc.tile_pool(name="sb", bufs=3) as sb,
            tc.tile_pool(name="ps", bufs=3, space="PSUM") as ps,
        ):
            et = cst.tile([P, 4, K], f32r)
            nc.sync.dma_start(et[:], etp[:, :, :].rearrange("a p n -> p a n"))
            ident = cst.tile([P, P], f32r)
            nc.sync.dma_start(ident[:], ident_d[:, :])
            ident_f32 = ident[:].bitcast(f32)

            for t in range(n_tiles):
                b, ti = divmod(t, TPB)
                c0 = ti * P
                # z tile: [C-sub(partition), half a, token] fp32r
                zt = sb.tile([P, 2, P], f32r, tag="zt")
                nc.sync.dma_start(
                    zt[:],
                    z[b, :, :].rearrange("(a p) m -> p a m", p=P)[:, :, c0 : c0 + P],
                )
                # scores: s = 2 z.e - |e|^2  -> argmax == argmin of distance
                sc = sb.tile([P, K], f32, tag="sc", bufs=2)
                for k in range(2):
                    pss = ps.tile([P, 512], f32, tag="sc")
                    nc.tensor.matmul(
                        pss[:], zt[:, 0, :], et[:, 0, k * 512 : (k + 1) * 512],
                        start=True, stop=False,
                    )
                    nc.tensor.matmul(
                        pss[:], zt[:, 1, :], et[:, 1, k * 512 : (k + 1) * 512],
                        start=False, stop=False,
                    )
                    nc.tensor.matmul(
                        pss[:], et[0:1, 3, 0:P], et[0:1, 2, k * 512 : (k + 1) * 512],
                        start=False, stop=True,
                    )
                    nc.scalar.copy(sc[:, k * 512 : (k + 1) * 512], pss[:])
                m8 = sb.tile([P, 8], f32, tag="m8")
                i8 = sb.tile([P, 8], u32, tag="i8")
                nc.vector.max(m8[:], sc[:])
                nc.vector.max_index(i8[:], m8[:], sc[:])
                # gather codebook rows
                zq = sb.tile([P, C], f32, tag="zq")
                nc.gpsimd.indirect_dma_start(
                    out=zq[:],
                    out_offset=None,
                    in_=etab[:, :],
                    in_offset=bass.IndirectOffsetOnAxis(ap=i8[:, 0:1], axis=0),
                )
                nc.sync.dma_start(zq_o[b, c0 : c0 + P, :], zq[:])
                # z_out = transpose(z_q) back to [C, HW]
                zoT = sb.tile([P, 2, P], f32, tag="zoT")
                for a in range(2):
                    pt = ps.tile([P, P], f32, tag=f"pta{a}", bufs=1)
                    nc.tensor.transpose(
                        pt[:], zq[:, a * P : (a + 1) * P], ident_f32
                    )
                    nc.scalar.copy(zoT[:, a, :], pt[:])
                nc.sync.dma_start(
                    zo_o[b, :, :].rearrange("(a p) m -> p a m", p=P)[:, :, c0 : c0 + P],
                    zoT[:],
                )
                # zp = transpose(z) to [HW, C] (values fp32r-rounded, ~1e-4 rel)
                zpt = sb.tile([P, 2, P], f32, tag="zpt")
                for a in range(2):
                    pt2 = ps.tile([P, P], f32r, tag=f"ptb{a}", bufs=1)
                    nc.tensor.transpose(pt2[:], zt[:, a, :], ident[:])
                    nc.scalar.copy(zpt[:, a, :], pt2[:].bitcast(f32))
                nc.sync.dma_start(zp_o[b, c0 : c0 + P, :], zpt[:])
    return nc


def prep_inputs(z_full, E):
    """Host-side prep: shard z, pack codebook operands."""
    z_full = np.ascontiguousarray(z_full, np.float32)
    E = np.ascontiguousarray(E, np.float32)
    ET2 = (2.0 * E.T).astype(np.float32)  # [C, K]
    e_sq = np.sum(E.astype(np.float64) * E.astype(np.float64), axis=1)
    etp = np.zeros((4, P, K), np.float32)
    etp[0] = ET2[:P]
    etp[1] = ET2[P:]
    etp[2, 0, :] = (-e_sq).astype(np.float32)
    etp[3, 0, :P] = 1.0
    ident = np.eye(P, dtype=np.float32)
    zr = z_full.reshape(NCORES, BPC, C, HW)
    in_maps = [
        dict(z=np.ascontiguousarray(zr[c]), etp=etp, etab=E, ident=ident)
        for c in range(NCORES)
    ]
    return in_maps


def assemble(results):
    """Gather per-core outputs into full tensors."""
    zq = np.concatenate([r["zq"] for r in results], axis=0)  # [16, HW, C]
    zo = np.concatenate([r["zo"] for r in results], axis=0)  # [16, C, HW]
    zp = np.concatenate([r["zp"] for r in results], axis=0)  # [16, HW, C]
    z_q = zq.reshape(B, H, W, C)
    zp_f = zp.reshape(B, H, W, C)
    z_out = zo.reshape(B, C, H, W)
    return z_out, z_q, zp_f


def run(z, embedding_weight, trace=False):
    from concourse.bass_utils import run_bass_kernel_spmd

    global _BUILT
    if _BUILT is None:
        _BUILT = build()
    in_maps = prep_inputs(z, embedding_weight)
    res = run_bass_kernel_spmd(
        _BUILT, in_maps, core_ids=list(range(NCORES)), trace=trace
    )
    return assemble(res.results), res


def kernel(z, embedding_weight):
    (z_out, z_q, zp), _ = run(z, embedding_weight, trace=False)
    return z_out, z_q, zp


# revision 12
# speedup vs baseline: 1.0669x; 1.0669x over previous
"""VQ codebook (vector-quantization) Trainium2 kernel.

Data-parallel over 8 NeuronCores: each core takes 2 of the 16 batches.
Per core: scores s[t,k] = 2*z_t.e_k - |e_k|^2 via fp32r PE matmuls (bias row
folded in as a K=1 matmul), argmax per token via DVE max/max_index, codebook
row gather via indirect DMA, and PE transposes to produce the NCHW outputs.

Returns (z_out, z_q, zp) matching the reference:
  zp    = transpose(z)  [B,H,W,C]
  z_q   = embedding[argmin_k ||z - e_k||^2]  [B,H,W,C]
  z_out = z_q laid out as NCHW (straight-through estimator is numerically z_q)
"""
import json
import numpy as np

B, C, H, W = 16, 256, 64, 64
K = 1024
NCORES = 8
BPC = B // NCORES  # batches per core
HW = H * W
P = 128
TPB = HW // P  # token tiles per batch
N_TILES = BPC * TPB  # token tiles per core

_BUILT = None


def _install_waitsplit():
    """walrus in this toolchain accepts at most ONE sync wait per instruction;
    split extra waits onto standalone EventSemaphore instructions."""
    from concourse.bass import Bass

    if getattr(Bass, "_waitsplit_installed", False):
        return
    Bass._waitsplit_installed = True
    orig = Bass.to_json_bytes

    def _split(bir):
        n = 0
        for fn in bir.get("functions", []):
            for blk in fn.get("blocks", []):
                out = []
                for ins in blk.get("instructions", []):
                    si = ins.get("sync_info")
                    ow = (si or {}).get("on_wait") or []
                    if len(ow) > 1:
                        for j, w in enumerate(ow[:-1]):
                            n += 1
                            out.append(
                                {
                                    "name": f"{ins['name']}-ws{j}",
                                    "opcode": "EventSemaphore",
                                    "engine": ins["engine"],
                                    "ins": [],
                                    "outs": [],
                                    "sync_info": {"on_wait": [w], "on_update": []},
                                }
                            )
                        si["on_wait"] = [ow[-1]]
                    out.append(ins)
                blk["instructions"] = out
        return n

    def patched(self, *a, **kw):
        m = json.loads(orig(self, *a, **kw))
        _split(m)
        return json.dumps(m).encode()

    Bass.to_json_bytes = patched


def build(n_tiles=N_TILES):
    import concourse.bass as bass
    import concourse.mybir as mybir
    import concourse.tile as tile

    _install_waitsplit()
    f32 = mybir.dt.float32
    f32r = mybir.dt.float32r
    u32 = mybir.dt.uint32

    bf16 = mybir.dt.bfloat16

    nc = bass.Bass()
    z = nc.dram_tensor("z", [BPC, C, HW], f32r, kind="ExternalInput")
    etp = nc.dram_tensor("etp", [4, P, K], f32r, kind="ExternalInput")
    etab = nc.dram_tensor("etab", [K, C], f32, kind="ExternalInput")
    ident_d = nc.dram_tensor("ident", [P, P], f32r, kind="ExternalInput")
    zq_o = nc.dram_tensor("zq", [BPC, HW, C], f32, kind="ExternalOutput")
    zo_o = nc.dram_tensor("zo", [BPC, C, HW], f32, kind="ExternalOutput")
    zp_o = nc.dram_tensor("zp", [BPC, HW, C], f32, kind="ExternalOutput")

    with tile.TileContext(nc) as tc:
        with (
            tc.tile_pool(name="const", bufs=1) as cst,
            tc.tile_pool(name="sb", bufs=4) as sb,
            tc.tile_pool(name="ps", bufs=4, space="PSUM") as ps,
        ):
            et = cst.tile([P, 4, K], f32r)
            nc.sync.dma_start(et[:], etp[:, :, :].rearrange("a p n -> p a n"))
            ident = cst.tile([P, P], f32r)
            nc.sync.dma_start(ident[:], ident_d[:, :])

            # bf16 warmup: ~20 dense matmuls flip the PE HAM clock-gate to
            # 8/8 (2.4 GHz); fp32r matmuls sustain it but don't trigger it.
            wt = cst.tile([P, 512], bf16)
            nc.vector.tensor_copy(wt[:], et[:, 0, 0:512].bitcast(f32))
            pw = ps.tile([P, 512], f32, tag="sc", bufs=4)
            for i in range(20):
                nc.tensor.matmul(
                    pw[:], wt[:, 0:128], wt[:, :], start=(i == 0), stop=(i == 19)
                )

            for t in range(n_tiles):
                b, ti = divmod(t, TPB)
                c0 = ti * P
                # z tile: [C-sub(partition), half a, token] fp32r
                zt = sb.tile([P, 2, P], f32r, tag="zt")
                nc.sync.dma_start(
                    zt[:],
                    z[b, :, :].rearrange("(a p) m -> p a m", p=P)[:, :, c0 : c0 + P],
                )
                # scores: s = 2 z.e - |e|^2  -> argmax == argmin of distance.
                # One weight-load per stationary operand (zt0, zt1, ones).
                sc = sb.tile([P, K], f32, tag="sc", bufs=3)
                pss = [
                    ps.tile([P, 512], f32, tag="sc", bufs=4, name=f"pss{k}")
                    for k in range(2)
                ]
                for w in range(3):
                    lhsT = et[0:1, 3, 0:P] if w == 2 else zt[:, w, :]
                    for k in range(2):
                        rhs = (
                            et[0:1, 2, k * 512 : (k + 1) * 512]
                            if w == 2
                            else et[:, w, k * 512 : (k + 1) * 512]
                        )
                        nc.tensor.matmul(
                            pss[k][:], lhsT, rhs, start=(w == 0), stop=(w == 2)
                        )
                for k in range(2):
                    nc.scalar.copy(sc[:, k * 512 : (k + 1) * 512], pss[k][:])
                m8 = sb.tile([P, 8], f32, tag="m8")
                i8 = sb.tile([P, 8], u32, tag="i8")
                nc.vector.max(m8[:], sc[:])
                nc.vector.max_index(i8[:], m8[:], sc[:])
                # gather codebook rows (exact fp32 bits)
                zq = sb.tile([P, C], f32, tag="zq")
                nc.gpsimd.indirect_dma_start(
                    out=zq[:],
                    out_offset=None,
                    in_=etab[:, :],
                    in_offset=bass.IndirectOffsetOnAxis(ap=i8[:, 0:1], axis=0),
                )
                nc.sync.dma_start(zq_o[b, c0 : c0 + P, :], zq[:])
                # transposes: zq -> z_out layout, zt -> zp layout; all four into
                # one PSUM tile (disjoint regions), drained by two ACT copies
                pt = ps.tile([P, 4, P], f32r, tag="pt", bufs=2)
                for a in range(2):
                    nc.tensor.transpose(
                        pt[:, a, :].bitcast(f32),
                        zq[:, a * P : (a + 1) * P],
                        ident[:].bitcast(f32),
                    )
                for a in range(2):
                    nc.tensor.transpose(pt[:, 2 + a, :], zt[:, a, :], ident[:])
                zoT = sb.tile([P, 2, P], f32, tag="zoT")
                zpt = sb.tile([P, 2, P], f32, tag="zpt")
                nc.scalar.copy(zoT[:], pt[:, 0:2, :].bitcast(f32))
                nc.scalar.copy(zpt[:], pt[:, 2:4, :].bitcast(f32))
                nc.sync.dma_start(
                    zo_o[b, :, :].rearrange("(a p) m -> p a m", p=P)[:, :, c0 : c0 + P],
                    zoT[:],
                )
                nc.sync.dma_start(zp_o[b, c0 : c0 + P, :], zpt[:])
    return nc


def prep_inputs(z_full, E):
    """Host-side prep: shard z, pack codebook operands."""
    z_full = np.ascontiguousarray(z_full, np.float32)
    E = np.ascontiguousarray(E, np.float32)
    ET2 = (2.0 * E.T).astype(np.float32)  # [C, K]
    e_sq = np.sum(E.astype(np.float64) * E.astype(np.float64), axis=1)
    etp = np.zeros((4, P, K), np.float32)
    etp[0] = ET2[:P]
    etp[1] = ET2[P:]
    etp[2, 0, :] = (-e_sq).astype(np.float32)
    etp[3, 0, :P] = 1.0
    ident = np.eye(P, dtype=np.float32)
    zr = z_full.reshape(NCORES, BPC, C, HW)
    in_maps = [
        dict(z=np.ascontiguousarray(zr[c]), etp=etp, etab=E, ident=ident)
        for c in range(NCORES)
    ]
    return in_maps


def assemble(results):
    """Gather per-core outputs into full tensors."""
    zq = np.concatenate([r["zq"] for r in results], axis=0)  # [16, HW, C]
    zo = np.concatenate([r["zo"] for r in results], axis=0)  # [16, C, HW]
    zp = np.concatenate([r["zp"] for r in results], axis=0)  # [16, HW, C]
    z_q = zq.reshape(B, H, W, C)
    zp_f = zp.reshape(B, H, W, C)
    z_out = zo.reshape(B, C, H, W)
    return z_out, z_q, zp_f


def run(z, embedding_weight, trace=False):
    from concourse.bass_utils import run_bass_kernel_spmd

    global _BUILT
    if _BUILT is None:
        _BUILT = build()
    in_maps = prep_inputs(z, embedding_weight)
    res = run_bass_kernel_spmd(
        _BUILT, in_maps, core_ids=list(range(NCORES)), trace=trace
    )
    return assemble(res.results), res


def kernel(z, embedding_weight):
    (z_out, z_q, zp), _ = run(z, embedding_weight, trace=False)
    return z_out, z_q, zp
